# revision 1
# baseline (speedup 1.0000x reference)
"""Canny filter Trainium2 Bass kernel (self-contained).

Row-sharded across 8 cores (128 rows of every image per core; the
batch-flattened NMS gather mixes images, so each core holds all 8 images
at its rows). Per-core "padded stack" per channel: 8 image blocks x 140
rows (6-row halos inline) x 1040 cols, tiled into 10 overlapping 128-row
tiles (stride 122). Vertical stencils: Toeplitz banded fp32 matmuls;
horizontal: shifted-rhs PSUM accumulation (13-tap 7x7 sobel-of-gaussian).
NMS: GPSIMD indirect_copy gathers over image-major interleaved GM tiles;
hysteresis: bf16 tridiagonal matmuls.
"""
import math
from contextlib import ExitStack

import numpy as np

import concourse.bass as bass
import concourse.bacc as bacc
import concourse.mybir as mybir
import concourse.tile as tile
from concourse.bass_utils import run_bass_kernel_spmd

mb = mybir
F32 = mb.dt.float32
BF16 = mb.dt.bfloat16
I32 = mb.dt.int32
U16 = mb.dt.uint16
ALU = mb.AluOpType
ACTF = mb.ActivationFunctionType

NCORES = 8
H = 1024
W = 1024
B = 8
C = 3
WP = 1040
WOFF = 8
BLK = 140
STACK = B * BLK
ROFFS = [0, 122, 244, 366, 488, 610, 732, 854, 976, 992]
CHUNKS = [(0, 512), (512, 512), (1024, 16)]
ROWS_PC = H // NCORES

QW = 128
QS = QW + 4
TW = B * QS  # 2080
TCHUNKS = [(0, 512), (512, 512), (1024, 32)]

T1, T2 = 10.0, 100.0
DIRS = [(0, 1), (1, 1), (1, 0), (1, -1), (0, -1), (-1, -1), (-1, 0), (-1, 1)]


def _filters():
    g = np.exp(-0.5 * (np.arange(5) - 2.0) ** 2).astype(np.float64)
    vg = np.convolve(g, [1.0, 2.0, 1.0])
    vd = np.convolve(g, [1.0, 0.0, -1.0])
    hd_eff = np.zeros(7)
    hg_eff = np.zeros(7)
    for k in range(5):
        hd_eff[(k - 2 - 1) + 3] += g[k]
        hd_eff[(k - 2 + 1) + 3] -= g[k]
        hg_eff[(k - 2 - 1) + 3] += g[k]
        hg_eff[(k - 2) + 3] += 2 * g[k]
        hg_eff[(k - 2 + 1) + 3] += g[k]
    return g, vg, vd, hd_eff, hg_eff


def _banded(prof, n=128):
    r = (len(prof) - 1) // 2
    m = np.zeros((n, n), np.float32)
    for o in range(n):
        for j in range(-r, r + 1):
            i = o + j
            if 0 <= i < n:
                m[i, o] = prof[j + r]
    return m


def _build(nc):
    g, vg, vd, hd_eff, hg_eff = _filters()
    BVG = _banded(vg)
    BVD = _banded(vd)

    img_d = nc.dram_tensor("img", [C, STACK, WP], F32, kind="ExternalInput")
    hmask_d = nc.dram_tensor("hmask", [STACK, 1], F32, kind="ExternalInput")
    edges_d = nc.dram_tensor("edges", [B, ROWS_PC, W], F32, kind="ExternalOutput")

    BLKS = 152
    gm_scr = nc.dram_tensor("gm_scr", [B, BLKS, WP], F32, kind="Internal")
    ip_scr = nc.dram_tensor("ip_scr", [B, BLKS, WP], U16, kind="Internal")

    # scaled banded matrices for every (filter, tap) — precomputed on host
    mats = []
    for j in range(-3, 4):
        if hd_eff[j + 3] != 0.0:
            mats.append(("x", j, np.float32(hd_eff[j + 3]) * BVG))
        if hg_eff[j + 3] != 0.0:
            mats.append(("y", j, np.float32(hg_eff[j + 3]) * BVD))
    allmats = np.stack([m for (_, _, m) in mats])  # [13,128,128]
    mats_t = nc.inline_tensor(
        np.ascontiguousarray(allmats.transpose(1, 0, 2).reshape(128, -1)), "mats"
    )  # [128, 13*128]

    wmask = np.zeros((128, WP), np.float32)
    wmask[:, WOFF : WOFF + W] = 1.0
    wmask_t = nc.inline_tensor(wmask, "wmask")
    bias4 = nc.inline_tensor(np.full((128, 1), 4.0, np.float32), "bias4")

    wv = np.zeros((128, TW), np.uint16)
    dwv = np.zeros((128, TW), np.int16)
    for b in range(B):
        for wt in range(QS):
            wv[:, QS * b + wt] = wt
            dwv[:, QS * b + wt] = DIRS[b][1]
    wv_t = nc.inline_tensor(wv, "wv")
    dwv_t = nc.inline_tensor(dwv.astype(np.uint16), "dwv")

    TRI_m = _banded([1.0, 1.0, 1.0]).astype(np.float32)
    TRI_x = np.zeros((16, 128), np.float32)
    TRI_x[0, 127] = 1.0
    TRI_xa = np.zeros((128, 16), np.float32)
    TRI_xb = np.zeros((16, 16), np.float32)
    for m2 in range(4):
        qq = 125 + m2
        for j in (-1, 0, 1):
            src = qq + j
            if src <= 125:
                if 0 <= src + 2 < 128:
                    TRI_xa[src + 2, m2] = 1.0
            else:
                if 0 <= src - 126 < 4:
                    TRI_xb[src - 126, m2] = 1.0
    import ml_dtypes
    tri_m_t = nc.inline_tensor(TRI_m.astype(ml_dtypes.bfloat16), "tri_m")
    tri_x_t = nc.inline_tensor(TRI_x.astype(ml_dtypes.bfloat16), "tri_x")
    tri_xa_t = nc.inline_tensor(TRI_xa.astype(ml_dtypes.bfloat16), "tri_xa")
    tri_xb_t = nc.inline_tensor(TRI_xb.astype(ml_dtypes.bfloat16), "tri_xb")

    with tile.TileContext(nc) as tc:
        with ExitStack() as octx:
            cpool = octx.enter_context(tc.tile_pool(name="consts", bufs=1))
            bias4_s = cpool.tile([128, 1], F32)
            nc.sync.dma_start(bias4_s[:], bias4.ap())

            # ---------------- stage 1 ----------------
            with ExitStack() as ctx:
                c1p = ctx.enter_context(tc.tile_pool(name="c1", bufs=1))
                inp = ctx.enter_context(tc.tile_pool(name="inp", bufs=2))
                work = ctx.enter_context(tc.tile_pool(name="work", bufs=1))
                small = ctx.enter_context(tc.tile_pool(name="small", bufs=2))
                psum = ctx.enter_context(
                    tc.tile_pool(name="psum", bufs=4, space="PSUM")
                )

                mats_s = c1p.tile([128, 13 * 128], F32)
                nc.sync.dma_start(mats_s[:], mats_t.ap())
                wmask_s = c1p.tile([128, WP], F32)
                nc.sync.dma_start(wmask_s[:], wmask_t.ap())
                hmask_s = c1p.tile([128, 10], F32)
                for t in range(10):
                    nc.sync.dma_start(
                        hmask_s[:, t : t + 1],
                        hmask_d[ROFFS[t] : ROFFS[t] + 128, :],
                    )

                def mat_ap(i):
                    return mats_s[:, 128 * i : 128 * (i + 1)]

                for t in range(10):
                    r0 = ROFFS[t]
                    gm = work.tile([128, WP], F32, tag="gm")
                    osum = work.tile([128, WP], F32, tag="osum")
                    suacc = work.tile([128, WP], F32, tag="suacc")
                    for c in range(C):
                        xin = inp.tile([128, WP], F32, tag="xin")
                        nc.sync.dma_start(xin[:], img_d[c, r0 : r0 + 128, :])
                        for (lo, n) in CHUNKS:
                            gxp = psum.tile([128, 512], F32, tag="gxp")
                            gyp = psum.tile([128, 512], F32, tag="gyp")
                            fx, fy = True, True
                            lastx = max(i for i, m in enumerate(mats) if m[0] == "x")
                            lasty = max(i for i, m in enumerate(mats) if m[0] == "y")
                            for mi, (kind, j, _) in enumerate(mats):
                                s, e = lo + j, lo + j + n
                                sc, ec = max(0, s), min(WP, e)
                                dst = (gxp if kind == "x" else gyp)[
                                    :, sc - s : n - (e - ec)
                                ]
                                nc.tensor.matmul(
                                    dst,
                                    mat_ap(mi),
                                    xin[:, sc:ec],
                                    start=(fx if kind == "x" else fy),
                                    stop=(mi == (lastx if kind == "x" else lasty)),
                                )
                                if kind == "x":
                                    fx = False
                                else:
                                    fy = False

                            sl = slice(lo, lo + n)
                            p2 = small.tile([128, 512], F32, tag="p2")
                            nc.scalar.square(p2[:, :n], gxp[:, :n])
                            q2 = small.tile([128, 512], F32, tag="q2")
                            nc.scalar.square(q2[:, :n], gyp[:, :n])
                            ss = small.tile([128, 512], F32, tag="ss")
                            nc.vector.tensor_tensor(
                                out=ss[:, :n], in0=p2[:, :n], in1=q2[:, :n],
                                op=ALU.add,
                            )
                            if c == 0:
                                nc.scalar.sqrt(gm[:, sl], ss[:, :n])
                            else:
                                rr = small.tile([128, 512], F32, tag="rr")
                                nc.scalar.sqrt(rr[:, :n], ss[:, :n])
                                nc.vector.tensor_tensor(
                                    out=gm[:, sl], in0=gm[:, sl],
                                    in1=rr[:, :n], op=ALU.add,
                                )
                            rc = small.tile([128, 512], F32, tag="rc")
                            nc.vector.reciprocal(rc[:, :n], gxp[:, :n])
                            qr = small.tile([128, 512], F32, tag="qr")
                            nc.vector.scalar_tensor_tensor(
                                out=qr[:, :n], in0=rc[:, :n], scalar=1.0,
                                in1=gyp[:, :n], op0=ALU.mult, op1=ALU.mult,
                            )
                            a0 = small.tile([128, 512], F32, tag="a0")
                            nc.scalar.activation(a0[:, :n], qr[:, :n], ACTF.Arctan)
                            su = small.tile([128, 512], F32, tag="su")
                            nc.vector.tensor_scalar(
                                out=su[:, :n], in0=gxp[:, :n], scalar1=0.0,
                                scalar2=None, op0=ALU.is_lt,
                            )
                            if c == 0:
                                nc.vector.tensor_copy(osum[:, sl], a0[:, :n])
                                nc.vector.tensor_copy(suacc[:, sl], su[:, :n])
                            else:
                                nc.vector.tensor_tensor(
                                    out=osum[:, sl], in0=osum[:, sl],
                                    in1=a0[:, :n], op=ALU.add,
                                )
                                nc.vector.tensor_tensor(
                                    out=suacc[:, sl], in0=suacc[:, sl],
                                    in1=su[:, :n], op=ALU.add,
                                )

                    gmm = work.tile([128, WP], F32, tag="gmm")
                    nc.vector.scalar_tensor_tensor(
                        out=gmm[:], in0=gm[:], scalar=hmask_s[:, t : t + 1],
                        in1=wmask_s[:], op0=ALU.mult, op1=ALU.mult,
                    )
                    zs = work.tile([128, WP], F32, tag="zs")
                    nc.scalar.activation(
                        zs[:], osum[:], ACTF.Identity, bias=bias4_s[:, 0:1],
                        scale=float(4.0 / math.pi),
                    )
                    z2 = work.tile([128, WP], F32, tag="z2")
                    nc.vector.scalar_tensor_tensor(
                        out=z2[:], in0=suacc[:], scalar=4.0, in1=zs[:],
                        op0=ALU.mult, op1=ALU.add,
                    )
                    zi = work.tile([128, WP], I32, tag="zi")
                    nc.vector.tensor_copy(zi[:], z2[:])
                    zm = work.tile([128, WP], I32, tag="zm")
                    nc.vector.tensor_scalar(
                        out=zm[:], in0=zi[:], scalar1=7, scalar2=None,
                        op0=ALU.bitwise_and,
                    )
                    ip16 = work.tile([128, WP], U16, tag="ip16")
                    nc.vector.tensor_copy(ip16[:], zm[:])

                    lo_r, hi_r = r0 + 3, r0 + 125
                    b0, b1 = lo_r // BLK, (hi_r - 1) // BLK
                    segs = [(lo_r, hi_r)] if b0 == b1 else [
                        (lo_r, (b0 + 1) * BLK), ((b0 + 1) * BLK, hi_r)]
                    for (s0, s1) in segs:
                        bb = s0 // BLK
                        pr0, pr1 = s0 - bb * BLK, s1 - bb * BLK
                        nc.sync.dma_start(
                            gm_scr[bb, pr0:pr1, :], gmm[s0 - r0 : s1 - r0, :]
                        )
                        nc.sync.dma_start(
                            ip_scr[bb, pr0:pr1, :], ip16[s0 - r0 : s1 - r0, :]
                        )

            # ---------------- stage 2: tail ----------------
            with ExitStack() as ctx:
                c2p = ctx.enter_context(tc.tile_pool(name="c2", bufs=1))
                tp = ctx.enter_context(tc.tile_pool(name="tail", bufs=1))
                tps = ctx.enter_context(
                    tc.tile_pool(name="tailps", bufs=2, space="PSUM")
                )

                tri_m_s = c2p.tile([128, 128], BF16)
                nc.sync.dma_start(tri_m_s[:], tri_m_t.ap())
                tri_x_s = c2p.tile([16, 128], BF16)
                nc.sync.dma_start(tri_x_s[:], tri_x_t.ap())
                tri_xa_s = c2p.tile([128, 16], BF16)
                nc.sync.dma_start(tri_xa_s[:], tri_xa_t.ap())
                tri_xb_s = c2p.tile([16, 16], BF16)
                nc.sync.dma_start(tri_xb_s[:], tri_xb_t.ap())

                for Q in range(8):
                    wp0 = WOFF + QW * Q - 2
                    gmi = {}
                    for v, dh in (("u", -1), ("c", 0), ("d", 1)):
                        gmain = tp.tile([128, TW], F32, tag=f"gmi{v}")
                        gx_ = tp.tile([16, TW], F32, tag=f"gmix{v}")
                        for bb in range(B):
                            nc.sync.dma_start(
                                gmain[:, QS * bb : QS * bb + QS],
                                gm_scr[bb, 4 + dh : 132 + dh, wp0 : wp0 + QS],
                            )
                            nc.sync.dma_start(
                                gx_[:, QS * bb : QS * bb + QS],
                                gm_scr[bb, 132 + dh : 148 + dh, wp0 : wp0 + QS],
                            )
                        gmi[v] = (gmain, gx_)
                    ipt_m = tp.tile([128, TW], U16, tag="iptm")
                    ipt_x = tp.tile([16, TW], U16, tag="iptx")
                    for bb in range(B):
                        nc.sync.dma_start(
                            ipt_m[:, QS * bb : QS * bb + QS],
                            ip_scr[bb, 4:132, wp0 : wp0 + QS],
                        )
                        nc.sync.dma_start(
                            ipt_x[:, QS * bb : QS * bb + QS],
                            ip_scr[bb, 132:148, wp0 : wp0 + QS],
                        )

                    def tail_chain(P, sfx, ipt, gset):
                        # masks from 2 low bits of idx (pair symmetry: only
                        # i+ mod 4 selects among pair-AND planes)
                        b0m = tp.tile([P, TW], U16, tag=f"ia{sfx}")
                        nc.vector.tensor_scalar(
                            out=b0m[:], in0=ipt[:], scalar1=1, scalar2=None,
                            op0=ALU.bitwise_and,
                        )
                        b1m = tp.tile([P, TW], U16, tag=f"ib{sfx}")
                        nc.vector.tensor_scalar(
                            out=b1m[:], in0=ipt[:], scalar1=1, scalar2=1,
                            op0=ALU.logical_shift_right, op1=ALU.bitwise_and,
                        )
                        gc, gu, gd = gset["c"], gset["u"], gset["d"]
                        ismax = tp.tile([P, TW], F32, tag=f"v1{sfx}")
                        ph = tp.tile([P, 4 * QS], F32, tag=f"v2{sfx}")
                        dd = tp.tile([P, TW], F32, tag=f"v3{sfx}")
                        for bb in range(B):
                            dh, dw = DIRS[bb]
                            var = gc if dh == 0 else (gd if dh == 1 else gu)
                            # D = GM > shift(GM): valid except block-edge slots
                            lo2 = max(0, -dw)
                            hi2 = TW - max(0, dw)
                            nc.vector.tensor_tensor(
                                out=dd[:, lo2:hi2], in0=gc[:, lo2:hi2],
                                in1=var[:, lo2 + dw : hi2 + dw], op=ALU.is_gt,
                            )
                            # pair AND: P[blk j] = D[blk j] * D[blk j+4], j<4
                            nc.vector.tensor_tensor(
                                out=ph[:], in0=dd[:, 0 : 4 * QS],
                                in1=dd[:, 4 * QS : 8 * QS], op=ALU.mult,
                            )
                            # 4-way select by (bit1, bit0) of idx at block bb
                            bsl = slice(QS * bb, QS * bb + QS)
                            ta = tp.tile([P, QS], F32, tag=f"ic{sfx}")
                            nc.vector.select(
                                ta[:], b0m[:, bsl], ph[:, QS : 2 * QS],
                                ph[:, 0:QS],
                            )
                            tb = tp.tile([P, QS], F32, tag=f"id{sfx}")
                            nc.vector.select(
                                tb[:], b0m[:, bsl], ph[:, 3 * QS : 4 * QS],
                                ph[:, 2 * QS : 3 * QS],
                            )
                            nc.vector.select(
                                ismax[:, bsl], b1m[:, bsl], tb[:], ta[:]
                            )
                        thin = tp.tile([P, TW], F32, tag=f"w4{sfx}")
                        nc.vector.tensor_tensor(
                            out=thin[:], in0=ismax[:], in1=gc[:], op=ALU.mult
                        )
                        return thin

                    thin_m = tail_chain(128, "m", ipt_m,
                                        {k: v[0] for k, v in gmi.items()})
                    thin_x = tail_chain(16, "x", ipt_x,
                                        {k: v[1] for k, v in gmi.items()})

                    high_m = tp.tile([128, TW], BF16, tag="highm")
                    nc.vector.tensor_scalar(
                        out=high_m[:], in0=thin_m[:], scalar1=T2, scalar2=None,
                        op0=ALU.is_gt,
                    )
                    high_x = tp.tile([16, TW], BF16, tag="highx")
                    nc.vector.tensor_scalar(
                        out=high_x[:], in0=thin_x[:], scalar1=T2, scalar2=None,
                        op0=ALU.is_gt,
                    )
                    vs_m = tp.tile([128, TW], F32, tag="w5m")
                    vs_x = tp.tile([16, TW], F32, tag="w5x")
                    for (lo, n) in TCHUNKS:
                        ps1 = tps.tile([128, 512], F32, tag="ps1")
                        nc.tensor.matmul(
                            ps1[:, :n], tri_m_s[:], high_m[:, lo : lo + n],
                            start=True, stop=False,
                        )
                        nc.tensor.matmul(
                            ps1[:, :n], tri_x_s[:], high_x[:, lo : lo + n],
                            start=False, stop=True,
                        )
                        nc.scalar.copy(vs_m[:, lo : lo + n], ps1[:, :n])
                        ps2 = tps.tile([16, 512], F32, tag="ps2")
                        nc.tensor.matmul(
                            ps2[:, :n], tri_xa_s[:], high_m[:, lo : lo + n],
                            start=True, stop=False,
                        )
                        nc.tensor.matmul(
                            ps2[:, :n], tri_xb_s[:], high_x[:, lo : lo + n],
                            start=False, stop=True,
                        )
                        nc.scalar.copy(vs_x[:, lo : lo + n], ps2[:, :n])

                    def finish(P, sfx, vs, thin, high):
                        h3 = tp.tile([P, TW], F32, tag=f"v2{sfx}")
                        nc.vector.tensor_tensor(
                            out=h3[:, 1 : TW - 1], in0=vs[:, 0 : TW - 2],
                            in1=vs[:, 2:TW], op=ALU.add,
                        )
                        c1t = tp.tile([P, TW], F32, tag=f"v3{sfx}")
                        nc.vector.tensor_tensor(
                            out=c1t[:, 1 : TW - 1], in0=h3[:, 1 : TW - 1],
                            in1=vs[:, 1 : TW - 1], op=ALU.add,
                        )
                        highf = tp.tile([P, TW], F32, tag=f"v4{sfx}")
                        nc.vector.tensor_copy(highf[:], high[:])
                        crgt = tp.tile([P, TW], F32, tag=f"w3{sfx}")
                        nc.vector.tensor_tensor(
                            out=crgt[:, 1 : TW - 1], in0=c1t[:, 1 : TW - 1],
                            in1=highf[:, 1 : TW - 1], op=ALU.is_gt,
                        )
                        m1 = tp.tile([P, TW], F32, tag=f"v1{sfx}")
                        nc.vector.tensor_scalar(
                            out=m1[:], in0=thin[:], scalar1=T1, scalar2=None,
                            op0=ALU.is_ge,
                        )
                        m2t = tp.tile([P, TW], F32, tag=f"w1{sfx}")
                        nc.vector.tensor_scalar(
                            out=m2t[:], in0=thin[:], scalar1=T2, scalar2=None,
                            op0=ALU.is_le,
                        )
                        mm_ = tp.tile([P, TW], F32, tag=f"w2{sfx}")
                        nc.vector.tensor_tensor(
                            out=mm_[:], in0=m1[:], in1=m2t[:], op=ALU.mult
                        )
                        t_ = tp.tile([P, TW], F32, tag=f"v2{sfx}")
                        nc.vector.tensor_tensor(
                            out=t_[:, 1 : TW - 1], in0=mm_[:, 1 : TW - 1],
                            in1=crgt[:, 1 : TW - 1], op=ALU.mult,
                        )
                        ed = tp.tile([P, TW], F32, tag=f"v3{sfx}")
                        nc.vector.tensor_tensor(
                            out=ed[:, 1 : TW - 1], in0=highf[:, 1 : TW - 1],
                            in1=t_[:, 1 : TW - 1], op=ALU.add,
                        )
                        return ed

                    ed_m = finish(128, "m", vs_m, thin_m, high_m)
                    ed_x = finish(16, "x", vs_x, thin_x, high_x)

                    for bb in range(B):
                        nc.sync.dma_start(
                            edges_d[bb, 0:126, QW * Q : QW * Q + QW],
                            ed_m[2:128, QS * bb + 2 : QS * bb + 2 + QW],
                        )
                        nc.sync.dma_start(
                            edges_d[bb, 126:128, QW * Q : QW * Q + QW],
                            ed_x[0:2, QS * bb + 2 : QS * bb + 2 + QW],
                        )


_COMPILED = {}


def _get_nc():
    if "nc" not in _COMPILED:
        nc = bacc.Bacc("TRN2", target_bir_lowering=False, debug=False,
                       num_devices=NCORES)
        _build(nc)
        nc.finalize()
        _COMPILED["nc"] = nc
    return _COMPILED["nc"]


def kernel(img: np.ndarray) -> np.ndarray:
    img = np.asarray(img, dtype=np.float32)
    assert img.shape == (B, C, H, W)
    nc = _get_nc()

    imgp = np.zeros((B, C, H + 12, WP), np.float32)
    imgp[:, :, 6 : 6 + H, WOFF : WOFF + W] = img

    in_maps = []
    for core in range(NCORES):
        r0 = ROWS_PC * core
        stack = np.empty((C, STACK, WP), np.float32)
        hm = np.zeros((STACK, 1), np.float32)
        for b in range(B):
            stack[:, b * BLK : (b + 1) * BLK, :] = imgp[b, :, r0 : r0 + BLK, :]
            for pr in range(BLK):
                gr = r0 + pr - 6
                hm[b * BLK + pr, 0] = 1.0 if 0 <= gr < H else 0.0
        in_maps.append({"img": stack, "hmask": hm})

    res = run_bass_kernel_spmd(nc, in_maps, core_ids=list(range(NCORES)))
    out = np.zeros((B, 1, H, W), np.float32)
    for core in range(NCORES):
        e = res.results[core]["edges"]
        out[:, 0, ROWS_PC * core : ROWS_PC * (core + 1), :] = e
    out[..., 0, :] = 0.0
    out[..., -1, :] = 0.0
    out[..., :, 0] = 0.0
    out[..., :, -1] = 0.0
    return out


if __name__ == "__main__":
    rng = np.random.RandomState(0)
    x = (rng.rand(B, C, H, W) * 255).astype(np.float32)
    y = kernel(x)
    print("out", y.shape, y.mean())



# revision 3
# speedup vs baseline: 3.4188x; 3.4188x over previous
"""Canny filter Trainium2 Bass kernel (self-contained).

Row-sharded across 8 cores (128 rows of every image per core; the
batch-flattened NMS gather mixes images, so each core holds all 8 images
at its rows). Per-core "padded stack" per channel: 8 image blocks x 140
rows (6-row halos inline) x 1024 cols, tiled into 10 overlapping 128-row
tiles (stride 122). Vertical stencils: Toeplitz banded fp32 matmuls;
horizontal: shifted-rhs PSUM accumulation (13-tap 7x7 sobel-of-gaussian).
Hysteresis: bf16 tridiagonal matmuls.

Wire-format optimizations (the axon tunnel moves ~75 MB/s, so bytes on
the wire dominate wall time):
  - input ships as uint16 fixed-point (img*256), converted to f32
    on-device; the 1/256 is folded into the stencil matrices and the
    256x into hmask.
  - output ships bit-packed: 8 image rows per byte via a power-of-two
    packing matmul, unpacked host-side with np.unpackbits.
  - the jitted SPMD executable is built once and cached (the stock
    run_bass_kernel_spmd path re-traces every call).
  - per-core input slabs are quantized on the host while previous slabs
    transfer, via a background device_put thread.
"""
import math
import queue
import threading
from contextlib import ExitStack

import numpy as np

import concourse.bacc as bacc
import concourse.mybir as mybir
import concourse.tile as tile

mb = mybir
F32 = mb.dt.float32
BF16 = mb.dt.bfloat16
I32 = mb.dt.int32
U16 = mb.dt.uint16
U8 = mb.dt.uint8
ALU = mb.AluOpType
ACTF = mb.ActivationFunctionType

NCORES = 8
H = 1024
W = 1024
B = 8
C = 3
WP = 1040
WOFF = 8
BLK = 140
STACK = B * BLK
ROFFS = [0, 122, 244, 366, 488, 610, 732, 854, 976, 992]
CHUNKS = [(0, 512), (512, 512), (1024, 16)]
ROWS_PC = H // NCORES

QW = 128
QS = QW + 4
TW = B * QS  # 1056
TCHUNKS = [(0, 512), (512, 512), (1024, 32)]

T1, T2 = 10.0, 100.0
DIRS = [(0, 1), (1, 1), (1, 0), (1, -1), (0, -1), (-1, -1), (-1, 0), (-1, 1)]

QSCALE = 256.0  # input fixed-point scale (u16 wire format)


def _filters():
    g = np.exp(-0.5 * (np.arange(5) - 2.0) ** 2).astype(np.float64)
    vg = np.convolve(g, [1.0, 2.0, 1.0])
    vd = np.convolve(g, [1.0, 0.0, -1.0])
    hd_eff = np.zeros(7)
    hg_eff = np.zeros(7)
    for k in range(5):
        hd_eff[(k - 2 - 1) + 3] += g[k]
        hd_eff[(k - 2 + 1) + 3] -= g[k]
        hg_eff[(k - 2 - 1) + 3] += g[k]
        hg_eff[(k - 2) + 3] += 2 * g[k]
        hg_eff[(k - 2 + 1) + 3] += g[k]
    return g, vg, vd, hd_eff, hg_eff


def _banded(prof, n=128):
    r = (len(prof) - 1) // 2
    m = np.zeros((n, n), np.float32)
    for o in range(n):
        for j in range(-r, r + 1):
            i = o + j
            if 0 <= i < n:
                m[i, o] = prof[j + r]
    return m


def _build(nc):
    g, vg, vd, hd_eff, hg_eff = _filters()
    BVG = _banded(vg)
    BVD = _banded(vd)

    img_d = nc.dram_tensor("img", [C, STACK, W], U16, kind="ExternalInput")
    hmask_d = nc.dram_tensor("hmask", [STACK, 1], F32, kind="ExternalInput")
    edges_d = nc.dram_tensor("edges", [B, 16, W], U8, kind="ExternalOutput")

    BLKS = 152
    gm_scr = nc.dram_tensor("gm_scr", [B, BLKS, WP], F32, kind="Internal")
    ip_scr = nc.dram_tensor("ip_scr", [B, BLKS, WP], U16, kind="Internal")

    # scaled banded matrices for every (filter, tap) — precomputed on host.
    # 1/QSCALE folds the u16 fixed-point unscale into the stencils.
    mats = []
    for j in range(-3, 4):
        if hd_eff[j + 3] != 0.0:
            mats.append(("x", j, np.float32(hd_eff[j + 3] / QSCALE) * BVG))
        if hg_eff[j + 3] != 0.0:
            mats.append(("y", j, np.float32(hg_eff[j + 3] / QSCALE) * BVD))
    allmats = np.stack([m for (_, _, m) in mats])  # [13,128,128]
    mats_t = nc.inline_tensor(
        np.ascontiguousarray(allmats.transpose(1, 0, 2).reshape(128, -1)), "mats"
    )  # [128, 13*128]

    wmask = np.zeros((128, WP), np.float32)
    wmask[:, WOFF : WOFF + W] = 1.0
    wmask_t = nc.inline_tensor(wmask, "wmask")
    bias4 = nc.inline_tensor(np.full((128, 1), 4.0, np.float32), "bias4")

    # row bit-packing matrices: out byte-row g = sum_k row(8g+k) << k.
    # ed_m rows 2..127 are output rows 0..125; ed_x rows 0..1 are 126..127.
    pack_m = np.zeros((128, 16), np.float32)
    for r in range(126):
        pack_m[2 + r, r // 8] = float(1 << (r % 8))
    pack_x = np.zeros((16, 16), np.float32)
    pack_x[0, 15] = 64.0
    pack_x[1, 15] = 128.0
    pack_m_t = nc.inline_tensor(pack_m, "pack_m")
    pack_x_t = nc.inline_tensor(pack_x, "pack_x")

    TRI_m = _banded([1.0, 1.0, 1.0]).astype(np.float32)
    TRI_x = np.zeros((16, 128), np.float32)
    TRI_x[0, 127] = 1.0
    TRI_xa = np.zeros((128, 16), np.float32)
    TRI_xb = np.zeros((16, 16), np.float32)
    for m2 in range(4):
        qq = 125 + m2
        for j in (-1, 0, 1):
            src = qq + j
            if src <= 125:
                if 0 <= src + 2 < 128:
                    TRI_xa[src + 2, m2] = 1.0
            else:
                if 0 <= src - 126 < 4:
                    TRI_xb[src - 126, m2] = 1.0
    import ml_dtypes
    tri_m_t = nc.inline_tensor(TRI_m.astype(ml_dtypes.bfloat16), "tri_m")
    tri_x_t = nc.inline_tensor(TRI_x.astype(ml_dtypes.bfloat16), "tri_x")
    tri_xa_t = nc.inline_tensor(TRI_xa.astype(ml_dtypes.bfloat16), "tri_xa")
    tri_xb_t = nc.inline_tensor(TRI_xb.astype(ml_dtypes.bfloat16), "tri_xb")

    with tile.TileContext(nc) as tc:
        with ExitStack() as octx:
            cpool = octx.enter_context(tc.tile_pool(name="consts", bufs=1))
            bias4_s = cpool.tile([128, 1], F32)
            nc.sync.dma_start(bias4_s[:], bias4.ap())

            # ---------------- stage 1 ----------------
            with ExitStack() as ctx:
                c1p = ctx.enter_context(tc.tile_pool(name="c1", bufs=1))
                inp16 = ctx.enter_context(tc.tile_pool(name="inp16", bufs=2))
                inp = ctx.enter_context(tc.tile_pool(name="inp", bufs=2))
                work = ctx.enter_context(tc.tile_pool(name="work", bufs=1))
                small = ctx.enter_context(tc.tile_pool(name="small", bufs=2))
                psum = ctx.enter_context(
                    tc.tile_pool(name="psum", bufs=4, space="PSUM")
                )

                mats_s = c1p.tile([128, 13 * 128], F32)
                nc.sync.dma_start(mats_s[:], mats_t.ap())
                wmask_s = c1p.tile([128, WP], F32)
                nc.sync.dma_start(wmask_s[:], wmask_t.ap())
                hmask_s = c1p.tile([128, 10], F32)
                for t in range(10):
                    nc.sync.dma_start(
                        hmask_s[:, t : t + 1],
                        hmask_d[ROFFS[t] : ROFFS[t] + 128, :],
                    )

                def mat_ap(i):
                    return mats_s[:, 128 * i : 128 * (i + 1)]

                for t in range(10):
                    r0 = ROFFS[t]
                    gm = work.tile([128, WP], F32, tag="gm")
                    osum = work.tile([128, WP], F32, tag="osum")
                    suacc = work.tile([128, WP], F32, tag="suacc")
                    for c in range(C):
                        xin16 = inp16.tile([128, W], U16, tag="xin16")
                        nc.sync.dma_start(xin16[:], img_d[c, r0 : r0 + 128, :])
                        xin = inp.tile([128, WP], F32, tag="xin")
                        nc.vector.memset(xin[:, 0:WOFF], 0)
                        nc.vector.memset(xin[:, WOFF + W : WP], 0)
                        nc.vector.tensor_copy(xin[:, WOFF : WOFF + W], xin16[:])
                        for (lo, n) in CHUNKS:
                            gxp = psum.tile([128, 512], F32, tag="gxp")
                            gyp = psum.tile([128, 512], F32, tag="gyp")
                            fx, fy = True, True
                            lastx = max(i for i, m in enumerate(mats) if m[0] == "x")
                            lasty = max(i for i, m in enumerate(mats) if m[0] == "y")
                            for mi, (kind, j, _) in enumerate(mats):
                                s, e = lo + j, lo + j + n
                                sc, ec = max(0, s), min(WP, e)
                                dst = (gxp if kind == "x" else gyp)[
                                    :, sc - s : n - (e - ec)
                                ]
                                nc.tensor.matmul(
                                    dst,
                                    mat_ap(mi),
                                    xin[:, sc:ec],
                                    start=(fx if kind == "x" else fy),
                                    stop=(mi == (lastx if kind == "x" else lasty)),
                                )
                                if kind == "x":
                                    fx = False
                                else:
                                    fy = False

                            sl = slice(lo, lo + n)
                            p2 = small.tile([128, 512], F32, tag="p2")
                            nc.scalar.square(p2[:, :n], gxp[:, :n])
                            q2 = small.tile([128, 512], F32, tag="q2")
                            nc.scalar.square(q2[:, :n], gyp[:, :n])
                            ss = small.tile([128, 512], F32, tag="ss")
                            nc.vector.tensor_tensor(
                                out=ss[:, :n], in0=p2[:, :n], in1=q2[:, :n],
                                op=ALU.add,
                            )
                            if c == 0:
                                nc.scalar.sqrt(gm[:, sl], ss[:, :n])
                            else:
                                rr = small.tile([128, 512], F32, tag="rr")
                                nc.scalar.sqrt(rr[:, :n], ss[:, :n])
                                nc.vector.tensor_tensor(
                                    out=gm[:, sl], in0=gm[:, sl],
                                    in1=rr[:, :n], op=ALU.add,
                                )
                            rc = small.tile([128, 512], F32, tag="rc")
                            nc.vector.reciprocal(rc[:, :n], gxp[:, :n])
                            qr = small.tile([128, 512], F32, tag="qr")
                            nc.vector.scalar_tensor_tensor(
                                out=qr[:, :n], in0=rc[:, :n], scalar=1.0,
                                in1=gyp[:, :n], op0=ALU.mult, op1=ALU.mult,
                            )
                            a0 = small.tile([128, 512], F32, tag="a0")
                            nc.scalar.activation(a0[:, :n], qr[:, :n], ACTF.Arctan)
                            su = small.tile([128, 512], F32, tag="su")
                            nc.vector.tensor_scalar(
                                out=su[:, :n], in0=gxp[:, :n], scalar1=0.0,
                                scalar2=None, op0=ALU.is_lt,
                            )
                            if c == 0:
                                nc.vector.tensor_copy(osum[:, sl], a0[:, :n])
                                nc.vector.tensor_copy(suacc[:, sl], su[:, :n])
                            else:
                                nc.vector.tensor_tensor(
                                    out=osum[:, sl], in0=osum[:, sl],
                                    in1=a0[:, :n], op=ALU.add,
                                )
                                nc.vector.tensor_tensor(
                                    out=suacc[:, sl], in0=suacc[:, sl],
                                    in1=su[:, :n], op=ALU.add,
                                )

                    gmm = work.tile([128, WP], F32, tag="gmm")
                    nc.vector.scalar_tensor_tensor(
                        out=gmm[:], in0=gm[:], scalar=hmask_s[:, t : t + 1],
                        in1=wmask_s[:], op0=ALU.mult, op1=ALU.mult,
                    )
                    zs = work.tile([128, WP], F32, tag="zs")
                    nc.scalar.activation(
                        zs[:], osum[:], ACTF.Identity, bias=bias4_s[:, 0:1],
                        scale=float(4.0 / math.pi),
                    )
                    z2 = work.tile([128, WP], F32, tag="z2")
                    nc.vector.scalar_tensor_tensor(
                        out=z2[:], in0=suacc[:], scalar=4.0, in1=zs[:],
                        op0=ALU.mult, op1=ALU.add,
                    )
                    zi = work.tile([128, WP], I32, tag="zi")
                    nc.vector.tensor_copy(zi[:], z2[:])
                    zm = work.tile([128, WP], I32, tag="zm")
                    nc.vector.tensor_scalar(
                        out=zm[:], in0=zi[:], scalar1=7, scalar2=None,
                        op0=ALU.bitwise_and,
                    )
                    ip16 = work.tile([128, WP], U16, tag="ip16")
                    nc.vector.tensor_copy(ip16[:], zm[:])

                    lo_r, hi_r = r0 + 3, r0 + 125
                    b0, b1 = lo_r // BLK, (hi_r - 1) // BLK
                    segs = [(lo_r, hi_r)] if b0 == b1 else [
                        (lo_r, (b0 + 1) * BLK), ((b0 + 1) * BLK, hi_r)]
                    for (s0, s1) in segs:
                        bb = s0 // BLK
                        pr0, pr1 = s0 - bb * BLK, s1 - bb * BLK
                        nc.sync.dma_start(
                            gm_scr[bb, pr0:pr1, :], gmm[s0 - r0 : s1 - r0, :]
                        )
                        nc.sync.dma_start(
                            ip_scr[bb, pr0:pr1, :], ip16[s0 - r0 : s1 - r0, :]
                        )

            # ---------------- stage 2: tail ----------------
            with ExitStack() as ctx:
                c2p = ctx.enter_context(tc.tile_pool(name="c2", bufs=1))
                tp = ctx.enter_context(tc.tile_pool(name="tail", bufs=1))
                tps = ctx.enter_context(
                    tc.tile_pool(name="tailps", bufs=2, space="PSUM")
                )

                tri_m_s = c2p.tile([128, 128], BF16)
                nc.sync.dma_start(tri_m_s[:], tri_m_t.ap())
                tri_x_s = c2p.tile([16, 128], BF16)
                nc.sync.dma_start(tri_x_s[:], tri_x_t.ap())
                tri_xa_s = c2p.tile([128, 16], BF16)
                nc.sync.dma_start(tri_xa_s[:], tri_xa_t.ap())
                tri_xb_s = c2p.tile([16, 16], BF16)
                nc.sync.dma_start(tri_xb_s[:], tri_xb_t.ap())
                pack_m_s = c2p.tile([128, 16], F32)
                nc.sync.dma_start(pack_m_s[:], pack_m_t.ap())
                pack_x_s = c2p.tile([16, 16], F32)
                nc.sync.dma_start(pack_x_s[:], pack_x_t.ap())

                for Q in range(8):
                    wp0 = WOFF + QW * Q - 2
                    gmi = {}
                    for v, dh in (("u", -1), ("c", 0), ("d", 1)):
                        gmain = tp.tile([128, TW], F32, tag=f"gmi{v}")
                        gx_ = tp.tile([16, TW], F32, tag=f"gmix{v}")
                        for bb in range(B):
                            nc.sync.dma_start(
                                gmain[:, QS * bb : QS * bb + QS],
                                gm_scr[bb, 4 + dh : 132 + dh, wp0 : wp0 + QS],
                            )
                            nc.sync.dma_start(
                                gx_[:, QS * bb : QS * bb + QS],
                                gm_scr[bb, 132 + dh : 148 + dh, wp0 : wp0 + QS],
                            )
                        gmi[v] = (gmain, gx_)
                    ipt_m = tp.tile([128, TW], U16, tag="iptm")
                    ipt_x = tp.tile([16, TW], U16, tag="iptx")
                    for bb in range(B):
                        nc.sync.dma_start(
                            ipt_m[:, QS * bb : QS * bb + QS],
                            ip_scr[bb, 4:132, wp0 : wp0 + QS],
                        )
                        nc.sync.dma_start(
                            ipt_x[:, QS * bb : QS * bb + QS],
                            ip_scr[bb, 132:148, wp0 : wp0 + QS],
                        )

                    def tail_chain(P, sfx, ipt, gset):
                        # masks from 2 low bits of idx (pair symmetry: only
                        # i+ mod 4 selects among pair-AND planes)
                        b0m = tp.tile([P, TW], U16, tag=f"ia{sfx}")
                        nc.vector.tensor_scalar(
                            out=b0m[:], in0=ipt[:], scalar1=1, scalar2=None,
                            op0=ALU.bitwise_and,
                        )
                        b1m = tp.tile([P, TW], U16, tag=f"ib{sfx}")
                        nc.vector.tensor_scalar(
                            out=b1m[:], in0=ipt[:], scalar1=1, scalar2=1,
                            op0=ALU.logical_shift_right, op1=ALU.bitwise_and,
                        )
                        gc, gu, gd = gset["c"], gset["u"], gset["d"]
                        ismax = tp.tile([P, TW], F32, tag=f"v1{sfx}")
                        ph = tp.tile([P, 4 * QS], F32, tag=f"v2{sfx}")
                        dd = tp.tile([P, TW], F32, tag=f"v3{sfx}")
                        for bb in range(B):
                            dh, dw = DIRS[bb]
                            var = gc if dh == 0 else (gd if dh == 1 else gu)
                            # D = GM > shift(GM): valid except block-edge slots
                            lo2 = max(0, -dw)
                            hi2 = TW - max(0, dw)
                            nc.vector.tensor_tensor(
                                out=dd[:, lo2:hi2], in0=gc[:, lo2:hi2],
                                in1=var[:, lo2 + dw : hi2 + dw], op=ALU.is_gt,
                            )
                            # pair AND: P[blk j] = D[blk j] * D[blk j+4], j<4
                            nc.vector.tensor_tensor(
                                out=ph[:], in0=dd[:, 0 : 4 * QS],
                                in1=dd[:, 4 * QS : 8 * QS], op=ALU.mult,
                            )
                            # 4-way select by (bit1, bit0) of idx at block bb
                            bsl = slice(QS * bb, QS * bb + QS)
                            ta = tp.tile([P, QS], F32, tag=f"ic{sfx}")
                            nc.vector.select(
                                ta[:], b0m[:, bsl], ph[:, QS : 2 * QS],
                                ph[:, 0:QS],
                            )
                            tb = tp.tile([P, QS], F32, tag=f"id{sfx}")
                            nc.vector.select(
                                tb[:], b0m[:, bsl], ph[:, 3 * QS : 4 * QS],
                                ph[:, 2 * QS : 3 * QS],
                            )
                            nc.vector.select(
                                ismax[:, bsl], b1m[:, bsl], tb[:], ta[:]
                            )
                        thin = tp.tile([P, TW], F32, tag=f"w4{sfx}")
                        nc.vector.tensor_tensor(
                            out=thin[:], in0=ismax[:], in1=gc[:], op=ALU.mult
                        )
                        return thin

                    thin_m = tail_chain(128, "m", ipt_m,
                                        {k: v[0] for k, v in gmi.items()})
                    thin_x = tail_chain(16, "x", ipt_x,
                                        {k: v[1] for k, v in gmi.items()})

                    high_m = tp.tile([128, TW], BF16, tag="highm")
                    nc.vector.tensor_scalar(
                        out=high_m[:], in0=thin_m[:], scalar1=T2, scalar2=None,
                        op0=ALU.is_gt,
                    )
                    high_x = tp.tile([16, TW], BF16, tag="highx")
                    nc.vector.tensor_scalar(
                        out=high_x[:], in0=thin_x[:], scalar1=T2, scalar2=None,
                        op0=ALU.is_gt,
                    )
                    vs_m = tp.tile([128, TW], F32, tag="w5m")
                    vs_x = tp.tile([16, TW], F32, tag="w5x")
                    for (lo, n) in TCHUNKS:
                        ps1 = tps.tile([128, 512], F32, tag="ps1")
                        nc.tensor.matmul(
                            ps1[:, :n], tri_m_s[:], high_m[:, lo : lo + n],
                            start=True, stop=False,
                        )
                        nc.tensor.matmul(
                            ps1[:, :n], tri_x_s[:], high_x[:, lo : lo + n],
                            start=False, stop=True,
                        )
                        nc.scalar.copy(vs_m[:, lo : lo + n], ps1[:, :n])
                        ps2 = tps.tile([16, 512], F32, tag="ps2")
                        nc.tensor.matmul(
                            ps2[:, :n], tri_xa_s[:], high_m[:, lo : lo + n],
                            start=True, stop=False,
                        )
                        nc.tensor.matmul(
                            ps2[:, :n], tri_xb_s[:], high_x[:, lo : lo + n],
                            start=False, stop=True,
                        )
                        nc.scalar.copy(vs_x[:, lo : lo + n], ps2[:, :n])

                    def finish(P, sfx, vs, thin, high):
                        h3 = tp.tile([P, TW], F32, tag=f"v2{sfx}")
                        nc.vector.tensor_tensor(
                            out=h3[:, 1 : TW - 1], in0=vs[:, 0 : TW - 2],
                            in1=vs[:, 2:TW], op=ALU.add,
                        )
                        c1t = tp.tile([P, TW], F32, tag=f"v3{sfx}")
                        nc.vector.tensor_tensor(
                            out=c1t[:, 1 : TW - 1], in0=h3[:, 1 : TW - 1],
                            in1=vs[:, 1 : TW - 1], op=ALU.add,
                        )
                        highf = tp.tile([P, TW], F32, tag=f"v4{sfx}")
                        nc.vector.tensor_copy(highf[:], high[:])
                        crgt = tp.tile([P, TW], F32, tag=f"w3{sfx}")
                        nc.vector.tensor_tensor(
                            out=crgt[:, 1 : TW - 1], in0=c1t[:, 1 : TW - 1],
                            in1=highf[:, 1 : TW - 1], op=ALU.is_gt,
                        )
                        m1 = tp.tile([P, TW], F32, tag=f"v1{sfx}")
                        nc.vector.tensor_scalar(
                            out=m1[:], in0=thin[:], scalar1=T1, scalar2=None,
                            op0=ALU.is_ge,
                        )
                        m2t = tp.tile([P, TW], F32, tag=f"w1{sfx}")
                        nc.vector.tensor_scalar(
                            out=m2t[:], in0=thin[:], scalar1=T2, scalar2=None,
                            op0=ALU.is_le,
                        )
                        mm_ = tp.tile([P, TW], F32, tag=f"w2{sfx}")
                        nc.vector.tensor_tensor(
                            out=mm_[:], in0=m1[:], in1=m2t[:], op=ALU.mult
                        )
                        t_ = tp.tile([P, TW], F32, tag=f"v2{sfx}")
                        nc.vector.tensor_tensor(
                            out=t_[:, 1 : TW - 1], in0=mm_[:, 1 : TW - 1],
                            in1=crgt[:, 1 : TW - 1], op=ALU.mult,
                        )
                        ed = tp.tile([P, TW], F32, tag=f"v3{sfx}")
                        nc.vector.tensor_tensor(
                            out=ed[:, 1 : TW - 1], in0=highf[:, 1 : TW - 1],
                            in1=t_[:, 1 : TW - 1], op=ALU.add,
                        )
                        return ed

                    ed_m = finish(128, "m", vs_m, thin_m, high_m)
                    ed_x = finish(16, "x", vs_x, thin_x, high_x)

                    # bit-pack 8 rows per byte: pk[g, w] = sum_k ed[8g+k, w]<<k
                    pk = tp.tile([16, TW], U8, tag="pk")
                    for (lo, n) in TCHUNKS:
                        psP = tps.tile([16, 512], F32, tag="psP")
                        nc.tensor.matmul(
                            psP[:, :n], pack_m_s[:], ed_m[:, lo : lo + n],
                            start=True, stop=False,
                        )
                        nc.tensor.matmul(
                            psP[:, :n], pack_x_s[:], ed_x[:, lo : lo + n],
                            start=False, stop=True,
                        )
                        nc.vector.tensor_copy(pk[:, lo : lo + n], psP[:, :n])

                    for bb in range(B):
                        nc.sync.dma_start(
                            edges_d[bb, :, QW * Q : QW * Q + QW],
                            pk[:, QS * bb + 2 : QS * bb + 2 + QW],
                        )


_CTX = {}


def _get_ctx():
    if _CTX:
        return _CTX
    import jax
    from jax.sharding import Mesh, PartitionSpec, NamedSharding
    from jax.experimental.shard_map import shard_map
    from concourse import bass2jax

    nc = bacc.Bacc("TRN2", target_bir_lowering=False, debug=False,
                   num_devices=NCORES)
    _build(nc)
    nc.finalize()
    bass2jax.install_neuronx_cc_hook()

    partition_name = (
        nc.partition_id_tensor.name if nc.partition_id_tensor else None
    )
    in_names, out_names, out_avals = [], [], []
    for alloc in nc.m.functions[0].allocations:
        if not isinstance(alloc, mybir.MemoryLocationSet):
            continue
        name = alloc.memorylocations[0].name
        if alloc.kind == "ExternalInput":
            if name != partition_name:
                in_names.append(name)
        elif alloc.kind == "ExternalOutput":
            out_names.append(name)
            shape = tuple(alloc.tensor_shape)
            dtype = mybir.dt.np(alloc.dtype)
            out_avals.append(jax.core.ShapedArray(shape, dtype))
    n_params = len(in_names)
    n_outs = len(out_names)
    bind_in_names = list(in_names) + list(out_names)
    if partition_name is not None:
        bind_in_names.append(partition_name)
    bind_in_names = tuple(bind_in_names)

    def _body(*args):
        operands = list(args)
        if partition_name is not None:
            operands.append(bass2jax.partition_id_tensor())
        outs = bass2jax._bass_exec_p.bind(
            *operands,
            out_avals=tuple(out_avals),
            in_names=bind_in_names,
            out_names=tuple(out_names),
            lowering_input_output_aliases=(),
            sim_require_finite=True,
            sim_require_nnan=True,
            nc=nc,
        )
        return tuple(outs)

    devices = jax.devices()[:NCORES]
    mesh = Mesh(np.asarray(devices), ("core",))
    P = PartitionSpec
    donate = tuple(range(n_params, n_params + n_outs))
    sharded = jax.jit(
        shard_map(
            _body, mesh=mesh,
            in_specs=(P("core"),) * (n_params + n_outs),
            out_specs=(P("core"),) * n_outs,
            check_rep=False,
        ),
        donate_argnums=donate, keep_unused=True,
    )

    # constant inputs, device-resident once
    core_sh = NamedSharding(mesh, P("core"))
    hm = np.zeros((NCORES * STACK, 1), np.float32)
    for core in range(NCORES):
        r0 = ROWS_PC * core
        for b in range(B):
            for pr in range(BLK):
                gr = r0 + pr - 6
                if 0 <= gr < H:
                    hm[core * STACK + b * BLK + pr, 0] = QSCALE
    hmask_dev = jax.device_put(hm, core_sh)
    hmask_dev.block_until_ready()

    dbg_zero = None
    if nc.dbg_addr is not None:
        dbg_zero = np.zeros((NCORES * 1, 2), np.uint32)

    _CTX.update(dict(
        jax=jax, nc=nc, mesh=mesh, core_sh=core_sh, devices=devices,
        sharded=sharded, in_names=in_names, out_names=out_names,
        n_params=n_params, hmask_dev=hmask_dev, dbg_zero=dbg_zero,
        dbg_name=nc.dbg_addr.name if nc.dbg_addr is not None else None,
        out_zero=np.zeros((NCORES * B, 16, W), np.uint8),
        slabs=[np.empty((C, STACK, W), np.uint16) for _ in range(NCORES)],
        tbuf=np.empty((B, C, BLK, W), np.float32),
    ))
    return _CTX


def _build_slab(ctx, img, core):
    """Quantize core's rows (img*256 -> u16) into stack layout."""
    t = ctx["tbuf"]
    lo = ROWS_PC * core - 6
    hi = lo + BLK
    s, e = max(lo, 0), min(hi, H)
    pa, pb = s - lo, e - lo
    if pa > 0:
        t[:, :, :pa, :] = 0.0
    if pb < BLK:
        t[:, :, pb:, :] = 0.0
    np.multiply(img[:, :, s:e, :], QSCALE, out=t[:, :, pa:pb, :])
    np.rint(t, out=t)
    slab = ctx["slabs"][core]
    slab.reshape(C, B, BLK, W)[:] = t.transpose(1, 0, 2, 3)
    return slab


def kernel(img: np.ndarray) -> np.ndarray:
    img = np.asarray(img, dtype=np.float32)
    assert img.shape == (B, C, H, W)
    ctx = _get_ctx()
    jax = ctx["jax"]
    devices = ctx["devices"]

    # build + ship per-core slabs; device_put runs on a worker thread so
    # quantization of slab c+1 overlaps the transfer of slab c.
    pieces = [None] * NCORES
    q = queue.Queue()

    def _xfer():
        while True:
            item = q.get()
            if item is None:
                return
            c, slab = item
            pieces[c] = jax.device_put(slab, devices[c])

    th = threading.Thread(target=_xfer)
    th.start()
    for core in range(NCORES):
        q.put((core, _build_slab(ctx, img, core)))
    q.put(None)
    th.join()

    img_arr = jax.make_array_from_single_device_arrays(
        (NCORES * C, STACK, W), ctx["core_sh"], pieces
    )

    args = []
    for name in ctx["in_names"]:
        if name == "img":
            args.append(img_arr)
        elif name == "hmask":
            args.append(ctx["hmask_dev"])
        elif name == ctx["dbg_name"]:
            args.append(ctx["dbg_zero"])
        else:
            raise KeyError(name)
    args.append(ctx["out_zero"])

    res = ctx["sharded"](*args)
    packed = np.asarray(res[0])  # [NCORES*B, 16, W] u8

    r = packed.reshape(NCORES, B, 16, W)
    bits = np.unpackbits(r, axis=2, bitorder="little")  # [NCORES,B,128,W]
    out = np.ascontiguousarray(bits.transpose(1, 0, 2, 3)).reshape(
        B, 1, H, W).astype(np.float32)
    out[..., 0, :] = 0.0
    out[..., -1, :] = 0.0
    out[..., :, 0] = 0.0
    out[..., :, -1] = 0.0
    return out


if __name__ == "__main__":
    rng = np.random.RandomState(0)
    x = (rng.rand(B, C, H, W) * 255).astype(np.float32)
    y = kernel(x)
    print("out", y.shape, y.mean())


# revision 17
# speedup vs baseline: 3.5929x; 1.0509x over previous
"""Canny filter Trainium2 Bass kernel (self-contained).

Row-sharded across 8 cores (128 rows of every image per core; the
batch-flattened NMS gather mixes images, so each core holds all 8 images
at its rows). Per-core "padded stack" per channel: 8 image blocks x 140
rows (6-row halos inline) x 1024 cols, tiled into 10 overlapping 128-row
tiles (stride 122). Vertical stencils: Toeplitz banded fp32 matmuls;
horizontal: shifted-rhs PSUM accumulation (13-tap 7x7 sobel-of-gaussian).
Hysteresis: bf16 tridiagonal matmuls.

Wire-format optimizations (the axon tunnel moves ~75 MB/s, so bytes on
the wire dominate wall time):
  - input ships as uint16 fixed-point (img*256), converted to f32
    on-device; the 1/256 is folded into the stencil matrices and the
    256x into hmask.
  - output ships bit-packed: 8 image rows per byte via a power-of-two
    packing matmul, unpacked host-side with np.unpackbits.
  - the jitted SPMD executable is built once and cached (the stock
    run_bass_kernel_spmd path re-traces every call).
  - per-core input slabs are quantized on the host while previous slabs
    transfer, via a background device_put thread.
"""
import math
import queue
import threading
from concurrent.futures import ThreadPoolExecutor
from contextlib import ExitStack

import numpy as np

import concourse.bacc as bacc
import concourse.mybir as mybir
import concourse.tile as tile

mb = mybir
F32 = mb.dt.float32
BF16 = mb.dt.bfloat16
I32 = mb.dt.int32
U16 = mb.dt.uint16
U8 = mb.dt.uint8
ALU = mb.AluOpType
ACTF = mb.ActivationFunctionType

NCORES = 8
H = 1024
W = 1024
B = 8
C = 3
WP = 1040
WOFF = 8
BLK = 140
STACK = B * BLK
ROFFS = [0, 122, 244, 366, 488, 610, 732, 854, 976, 992]
CHUNKS = [(0, 512), (512, 512), (1024, 16)]
ROWS_PC = H // NCORES

QW = 128
QS = QW + 4
TW = B * QS  # 1056
TCHUNKS = [(0, 512), (512, 512), (1024, 32)]

T1, T2 = 10.0, 100.0
DIRS = [(0, 1), (1, 1), (1, 0), (1, -1), (0, -1), (-1, -1), (-1, 0), (-1, 1)]

QSCALE = 256.0  # input fixed-point scale (u16 wire format)


def _filters():
    g = np.exp(-0.5 * (np.arange(5) - 2.0) ** 2).astype(np.float64)
    vg = np.convolve(g, [1.0, 2.0, 1.0])
    vd = np.convolve(g, [1.0, 0.0, -1.0])
    hd_eff = np.zeros(7)
    hg_eff = np.zeros(7)
    for k in range(5):
        hd_eff[(k - 2 - 1) + 3] += g[k]
        hd_eff[(k - 2 + 1) + 3] -= g[k]
        hg_eff[(k - 2 - 1) + 3] += g[k]
        hg_eff[(k - 2) + 3] += 2 * g[k]
        hg_eff[(k - 2 + 1) + 3] += g[k]
    return g, vg, vd, hd_eff, hg_eff


def _banded(prof, n=128):
    r = (len(prof) - 1) // 2
    m = np.zeros((n, n), np.float32)
    for o in range(n):
        for j in range(-r, r + 1):
            i = o + j
            if 0 <= i < n:
                m[i, o] = prof[j + r]
    return m


def _build(nc):
    g, vg, vd, hd_eff, hg_eff = _filters()
    BVG = _banded(vg)
    BVD = _banded(vd)

    img_d = nc.dram_tensor("img", [C, STACK, W], U16, kind="ExternalInput")
    hmask_d = nc.dram_tensor("hmask", [STACK, 1], F32, kind="ExternalInput")
    edges_d = nc.dram_tensor("edges", [B, 16, W], U8, kind="ExternalOutput")

    BLKS = 152
    gm_scr = nc.dram_tensor("gm_scr", [B, BLKS, WP], F32, kind="Internal")
    ip_scr = nc.dram_tensor("ip_scr", [B, BLKS, WP], U16, kind="Internal")

    # scaled banded matrices for every (filter, tap) — precomputed on host.
    # 1/QSCALE folds the u16 fixed-point unscale into the stencils.
    mats = []
    for j in range(-3, 4):
        if hd_eff[j + 3] != 0.0:
            mats.append(("x", j, np.float32(hd_eff[j + 3] / QSCALE) * BVG))
        if hg_eff[j + 3] != 0.0:
            mats.append(("y", j, np.float32(hg_eff[j + 3] / QSCALE) * BVD))
    allmats = np.stack([m for (_, _, m) in mats])  # [13,128,128]
    mats_t = nc.inline_tensor(
        np.ascontiguousarray(allmats.transpose(1, 0, 2).reshape(128, -1)), "mats"
    )  # [128, 13*128]

    wmask = np.zeros((128, WP), np.float32)
    wmask[:, WOFF : WOFF + W] = 1.0
    wmask_t = nc.inline_tensor(wmask, "wmask")
    bias4 = nc.inline_tensor(np.full((128, 1), 4.0, np.float32), "bias4")

    # row bit-packing matrices: out byte-row g = sum_k row(8g+k) << k.
    # ed_m rows 2..127 are output rows 0..125; ed_x rows 0..1 are 126..127.
    pack_m = np.zeros((128, 16), np.float32)
    for r in range(126):
        pack_m[2 + r, r // 8] = float(1 << (r % 8))
    pack_x = np.zeros((16, 16), np.float32)
    pack_x[0, 15] = 64.0
    pack_x[1, 15] = 128.0
    pack_m_t = nc.inline_tensor(pack_m, "pack_m")
    pack_x_t = nc.inline_tensor(pack_x, "pack_x")

    TRI_m = _banded([1.0, 1.0, 1.0]).astype(np.float32)
    TRI_x = np.zeros((16, 128), np.float32)
    TRI_x[0, 127] = 1.0
    TRI_xa = np.zeros((128, 16), np.float32)
    TRI_xb = np.zeros((16, 16), np.float32)
    for m2 in range(4):
        qq = 125 + m2
        for j in (-1, 0, 1):
            src = qq + j
            if src <= 125:
                if 0 <= src + 2 < 128:
                    TRI_xa[src + 2, m2] = 1.0
            else:
                if 0 <= src - 126 < 4:
                    TRI_xb[src - 126, m2] = 1.0
    import ml_dtypes
    tri_m_t = nc.inline_tensor(TRI_m.astype(ml_dtypes.bfloat16), "tri_m")
    tri_x_t = nc.inline_tensor(TRI_x.astype(ml_dtypes.bfloat16), "tri_x")
    tri_xa_t = nc.inline_tensor(TRI_xa.astype(ml_dtypes.bfloat16), "tri_xa")
    tri_xb_t = nc.inline_tensor(TRI_xb.astype(ml_dtypes.bfloat16), "tri_xb")

    with tile.TileContext(nc) as tc:
        with ExitStack() as octx:
            cpool = octx.enter_context(tc.tile_pool(name="consts", bufs=1))
            bias4_s = cpool.tile([128, 1], F32)
            nc.sync.dma_start(bias4_s[:], bias4.ap())

            # ---------------- stage 1 ----------------
            with ExitStack() as ctx:
                c1p = ctx.enter_context(tc.tile_pool(name="c1", bufs=1))
                inp16 = ctx.enter_context(tc.tile_pool(name="inp16", bufs=2))
                inp = ctx.enter_context(tc.tile_pool(name="inp", bufs=2))
                work = ctx.enter_context(tc.tile_pool(name="work", bufs=1))
                small = ctx.enter_context(tc.tile_pool(name="small", bufs=2))
                psum = ctx.enter_context(
                    tc.tile_pool(name="psum", bufs=4, space="PSUM")
                )

                mats_s = c1p.tile([128, 13 * 128], F32)
                nc.sync.dma_start(mats_s[:], mats_t.ap())
                wmask_s = c1p.tile([128, WP], F32)
                nc.sync.dma_start(wmask_s[:], wmask_t.ap())
                hmask_s = c1p.tile([128, 10], F32)
                for t in range(10):
                    nc.sync.dma_start(
                        hmask_s[:, t : t + 1],
                        hmask_d[ROFFS[t] : ROFFS[t] + 128, :],
                    )

                def mat_ap(i):
                    return mats_s[:, 128 * i : 128 * (i + 1)]

                for t in range(10):
                    r0 = ROFFS[t]
                    gm = work.tile([128, WP], F32, tag="gm")
                    osum = work.tile([128, WP], F32, tag="osum")
                    suacc = work.tile([128, WP], F32, tag="suacc")
                    for c in range(C):
                        xin16 = inp16.tile([128, W], U16, tag="xin16")
                        nc.sync.dma_start(xin16[:], img_d[c, r0 : r0 + 128, :])
                        xin = inp.tile([128, WP], F32, tag="xin")
                        nc.vector.memset(xin[:, 0:WOFF], 0)
                        nc.vector.memset(xin[:, WOFF + W : WP], 0)
                        nc.vector.tensor_copy(xin[:, WOFF : WOFF + W], xin16[:])
                        for (lo, n) in CHUNKS:
                            gxp = psum.tile([128, 512], F32, tag="gxp")
                            gyp = psum.tile([128, 512], F32, tag="gyp")
                            fx, fy = True, True
                            lastx = max(i for i, m in enumerate(mats) if m[0] == "x")
                            lasty = max(i for i, m in enumerate(mats) if m[0] == "y")
                            for mi, (kind, j, _) in enumerate(mats):
                                s, e = lo + j, lo + j + n
                                sc, ec = max(0, s), min(WP, e)
                                dst = (gxp if kind == "x" else gyp)[
                                    :, sc - s : n - (e - ec)
                                ]
                                nc.tensor.matmul(
                                    dst,
                                    mat_ap(mi),
                                    xin[:, sc:ec],
                                    start=(fx if kind == "x" else fy),
                                    stop=(mi == (lastx if kind == "x" else lasty)),
                                )
                                if kind == "x":
                                    fx = False
                                else:
                                    fy = False

                            sl = slice(lo, lo + n)
                            p2 = small.tile([128, 512], F32, tag="p2")
                            nc.scalar.square(p2[:, :n], gxp[:, :n])
                            q2 = small.tile([128, 512], F32, tag="q2")
                            nc.scalar.square(q2[:, :n], gyp[:, :n])
                            ss = small.tile([128, 512], F32, tag="ss")
                            nc.vector.tensor_tensor(
                                out=ss[:, :n], in0=p2[:, :n], in1=q2[:, :n],
                                op=ALU.add,
                            )
                            if c == 0:
                                nc.scalar.sqrt(gm[:, sl], ss[:, :n])
                            else:
                                rr = small.tile([128, 512], F32, tag="rr")
                                nc.scalar.sqrt(rr[:, :n], ss[:, :n])
                                nc.vector.tensor_tensor(
                                    out=gm[:, sl], in0=gm[:, sl],
                                    in1=rr[:, :n], op=ALU.add,
                                )
                            rc = small.tile([128, 512], F32, tag="rc")
                            nc.vector.reciprocal(rc[:, :n], gxp[:, :n])
                            qr = small.tile([128, 512], F32, tag="qr")
                            nc.vector.scalar_tensor_tensor(
                                out=qr[:, :n], in0=rc[:, :n], scalar=1.0,
                                in1=gyp[:, :n], op0=ALU.mult, op1=ALU.mult,
                            )
                            a0 = small.tile([128, 512], F32, tag="a0")
                            nc.scalar.activation(a0[:, :n], qr[:, :n], ACTF.Arctan)
                            su = small.tile([128, 512], F32, tag="su")
                            nc.vector.tensor_scalar(
                                out=su[:, :n], in0=gxp[:, :n], scalar1=0.0,
                                scalar2=None, op0=ALU.is_lt,
                            )
                            if c == 0:
                                nc.vector.tensor_copy(osum[:, sl], a0[:, :n])
                                nc.vector.tensor_copy(suacc[:, sl], su[:, :n])
                            else:
                                nc.vector.tensor_tensor(
                                    out=osum[:, sl], in0=osum[:, sl],
                                    in1=a0[:, :n], op=ALU.add,
                                )
                                nc.vector.tensor_tensor(
                                    out=suacc[:, sl], in0=suacc[:, sl],
                                    in1=su[:, :n], op=ALU.add,
                                )

                    gmm = work.tile([128, WP], F32, tag="gmm")
                    nc.vector.scalar_tensor_tensor(
                        out=gmm[:], in0=gm[:], scalar=hmask_s[:, t : t + 1],
                        in1=wmask_s[:], op0=ALU.mult, op1=ALU.mult,
                    )
                    zs = work.tile([128, WP], F32, tag="zs")
                    nc.scalar.activation(
                        zs[:], osum[:], ACTF.Identity, bias=bias4_s[:, 0:1],
                        scale=float(4.0 / math.pi),
                    )
                    z2 = work.tile([128, WP], F32, tag="z2")
                    nc.vector.scalar_tensor_tensor(
                        out=z2[:], in0=suacc[:], scalar=4.0, in1=zs[:],
                        op0=ALU.mult, op1=ALU.add,
                    )
                    zi = work.tile([128, WP], I32, tag="zi")
                    nc.vector.tensor_copy(zi[:], z2[:])
                    zm = work.tile([128, WP], I32, tag="zm")
                    nc.vector.tensor_scalar(
                        out=zm[:], in0=zi[:], scalar1=7, scalar2=None,
                        op0=ALU.bitwise_and,
                    )
                    ip16 = work.tile([128, WP], U16, tag="ip16")
                    nc.vector.tensor_copy(ip16[:], zm[:])

                    lo_r, hi_r = r0 + 3, r0 + 125
                    b0, b1 = lo_r // BLK, (hi_r - 1) // BLK
                    segs = [(lo_r, hi_r)] if b0 == b1 else [
                        (lo_r, (b0 + 1) * BLK), ((b0 + 1) * BLK, hi_r)]
                    for (s0, s1) in segs:
                        bb = s0 // BLK
                        pr0, pr1 = s0 - bb * BLK, s1 - bb * BLK
                        nc.sync.dma_start(
                            gm_scr[bb, pr0:pr1, :], gmm[s0 - r0 : s1 - r0, :]
                        )
                        nc.sync.dma_start(
                            ip_scr[bb, pr0:pr1, :], ip16[s0 - r0 : s1 - r0, :]
                        )

            # ---------------- stage 2: tail ----------------
            with ExitStack() as ctx:
                c2p = ctx.enter_context(tc.tile_pool(name="c2", bufs=1))
                tp = ctx.enter_context(tc.tile_pool(name="tail", bufs=1))
                tps = ctx.enter_context(
                    tc.tile_pool(name="tailps", bufs=2, space="PSUM")
                )

                tri_m_s = c2p.tile([128, 128], BF16)
                nc.sync.dma_start(tri_m_s[:], tri_m_t.ap())
                tri_x_s = c2p.tile([16, 128], BF16)
                nc.sync.dma_start(tri_x_s[:], tri_x_t.ap())
                tri_xa_s = c2p.tile([128, 16], BF16)
                nc.sync.dma_start(tri_xa_s[:], tri_xa_t.ap())
                tri_xb_s = c2p.tile([16, 16], BF16)
                nc.sync.dma_start(tri_xb_s[:], tri_xb_t.ap())
                pack_m_s = c2p.tile([128, 16], F32)
                nc.sync.dma_start(pack_m_s[:], pack_m_t.ap())
                pack_x_s = c2p.tile([16, 16], F32)
                nc.sync.dma_start(pack_x_s[:], pack_x_t.ap())

                for Q in range(8):
                    wp0 = WOFF + QW * Q - 2
                    gmi = {}
                    for v, dh in (("u", -1), ("c", 0), ("d", 1)):
                        gmain = tp.tile([128, TW], F32, tag=f"gmi{v}")
                        gx_ = tp.tile([16, TW], F32, tag=f"gmix{v}")
                        for bb in range(B):
                            nc.sync.dma_start(
                                gmain[:, QS * bb : QS * bb + QS],
                                gm_scr[bb, 4 + dh : 132 + dh, wp0 : wp0 + QS],
                            )
                            nc.sync.dma_start(
                                gx_[:, QS * bb : QS * bb + QS],
                                gm_scr[bb, 132 + dh : 148 + dh, wp0 : wp0 + QS],
                            )
                        gmi[v] = (gmain, gx_)
                    ipt_m = tp.tile([128, TW], U16, tag="iptm")
                    ipt_x = tp.tile([16, TW], U16, tag="iptx")
                    for bb in range(B):
                        nc.sync.dma_start(
                            ipt_m[:, QS * bb : QS * bb + QS],
                            ip_scr[bb, 4:132, wp0 : wp0 + QS],
                        )
                        nc.sync.dma_start(
                            ipt_x[:, QS * bb : QS * bb + QS],
                            ip_scr[bb, 132:148, wp0 : wp0 + QS],
                        )

                    def tail_chain(P, sfx, ipt, gset):
                        # masks from 2 low bits of idx (pair symmetry: only
                        # i+ mod 4 selects among pair-AND planes)
                        b0m = tp.tile([P, TW], U16, tag=f"ia{sfx}")
                        nc.vector.tensor_scalar(
                            out=b0m[:], in0=ipt[:], scalar1=1, scalar2=None,
                            op0=ALU.bitwise_and,
                        )
                        b1m = tp.tile([P, TW], U16, tag=f"ib{sfx}")
                        nc.vector.tensor_scalar(
                            out=b1m[:], in0=ipt[:], scalar1=1, scalar2=1,
                            op0=ALU.logical_shift_right, op1=ALU.bitwise_and,
                        )
                        gc, gu, gd = gset["c"], gset["u"], gset["d"]
                        ismax = tp.tile([P, TW], F32, tag=f"v1{sfx}")
                        ph = tp.tile([P, 4 * QS], F32, tag=f"v2{sfx}")
                        dd = tp.tile([P, TW], F32, tag=f"v3{sfx}")
                        for bb in range(B):
                            dh, dw = DIRS[bb]
                            var = gc if dh == 0 else (gd if dh == 1 else gu)
                            # D = GM > shift(GM): valid except block-edge slots
                            lo2 = max(0, -dw)
                            hi2 = TW - max(0, dw)
                            nc.vector.tensor_tensor(
                                out=dd[:, lo2:hi2], in0=gc[:, lo2:hi2],
                                in1=var[:, lo2 + dw : hi2 + dw], op=ALU.is_gt,
                            )
                            # pair AND: P[blk j] = D[blk j] * D[blk j+4], j<4
                            nc.vector.tensor_tensor(
                                out=ph[:], in0=dd[:, 0 : 4 * QS],
                                in1=dd[:, 4 * QS : 8 * QS], op=ALU.mult,
                            )
                            # 4-way select by (bit1, bit0) of idx at block bb
                            bsl = slice(QS * bb, QS * bb + QS)
                            ta = tp.tile([P, QS], F32, tag=f"ic{sfx}")
                            nc.vector.select(
                                ta[:], b0m[:, bsl], ph[:, QS : 2 * QS],
                                ph[:, 0:QS],
                            )
                            tb = tp.tile([P, QS], F32, tag=f"id{sfx}")
                            nc.vector.select(
                                tb[:], b0m[:, bsl], ph[:, 3 * QS : 4 * QS],
                                ph[:, 2 * QS : 3 * QS],
                            )
                            nc.vector.select(
                                ismax[:, bsl], b1m[:, bsl], tb[:], ta[:]
                            )
                        thin = tp.tile([P, TW], F32, tag=f"w4{sfx}")
                        nc.vector.tensor_tensor(
                            out=thin[:], in0=ismax[:], in1=gc[:], op=ALU.mult
                        )
                        return thin

                    thin_m = tail_chain(128, "m", ipt_m,
                                        {k: v[0] for k, v in gmi.items()})
                    thin_x = tail_chain(16, "x", ipt_x,
                                        {k: v[1] for k, v in gmi.items()})

                    high_m = tp.tile([128, TW], BF16, tag="highm")
                    nc.vector.tensor_scalar(
                        out=high_m[:], in0=thin_m[:], scalar1=T2, scalar2=None,
                        op0=ALU.is_gt,
                    )
                    high_x = tp.tile([16, TW], BF16, tag="highx")
                    nc.vector.tensor_scalar(
                        out=high_x[:], in0=thin_x[:], scalar1=T2, scalar2=None,
                        op0=ALU.is_gt,
                    )
                    vs_m = tp.tile([128, TW], F32, tag="w5m")
                    vs_x = tp.tile([16, TW], F32, tag="w5x")
                    for (lo, n) in TCHUNKS:
                        ps1 = tps.tile([128, 512], F32, tag="ps1")
                        nc.tensor.matmul(
                            ps1[:, :n], tri_m_s[:], high_m[:, lo : lo + n],
                            start=True, stop=False,
                        )
                        nc.tensor.matmul(
                            ps1[:, :n], tri_x_s[:], high_x[:, lo : lo + n],
                            start=False, stop=True,
                        )
                        nc.scalar.copy(vs_m[:, lo : lo + n], ps1[:, :n])
                        ps2 = tps.tile([16, 512], F32, tag="ps2")
                        nc.tensor.matmul(
                            ps2[:, :n], tri_xa_s[:], high_m[:, lo : lo + n],
                            start=True, stop=False,
                        )
                        nc.tensor.matmul(
                            ps2[:, :n], tri_xb_s[:], high_x[:, lo : lo + n],
                            start=False, stop=True,
                        )
                        nc.scalar.copy(vs_x[:, lo : lo + n], ps2[:, :n])

                    def finish(P, sfx, vs, thin, high):
                        h3 = tp.tile([P, TW], F32, tag=f"v2{sfx}")
                        nc.vector.tensor_tensor(
                            out=h3[:, 1 : TW - 1], in0=vs[:, 0 : TW - 2],
                            in1=vs[:, 2:TW], op=ALU.add,
                        )
                        c1t = tp.tile([P, TW], F32, tag=f"v3{sfx}")
                        nc.vector.tensor_tensor(
                            out=c1t[:, 1 : TW - 1], in0=h3[:, 1 : TW - 1],
                            in1=vs[:, 1 : TW - 1], op=ALU.add,
                        )
                        highf = tp.tile([P, TW], F32, tag=f"v4{sfx}")
                        nc.vector.tensor_copy(highf[:], high[:])
                        crgt = tp.tile([P, TW], F32, tag=f"w3{sfx}")
                        nc.vector.tensor_tensor(
                            out=crgt[:, 1 : TW - 1], in0=c1t[:, 1 : TW - 1],
                            in1=highf[:, 1 : TW - 1], op=ALU.is_gt,
                        )
                        m1 = tp.tile([P, TW], F32, tag=f"v1{sfx}")
                        nc.vector.tensor_scalar(
                            out=m1[:], in0=thin[:], scalar1=T1, scalar2=None,
                            op0=ALU.is_ge,
                        )
                        m2t = tp.tile([P, TW], F32, tag=f"w1{sfx}")
                        nc.vector.tensor_scalar(
                            out=m2t[:], in0=thin[:], scalar1=T2, scalar2=None,
                            op0=ALU.is_le,
                        )
                        mm_ = tp.tile([P, TW], F32, tag=f"w2{sfx}")
                        nc.vector.tensor_tensor(
                            out=mm_[:], in0=m1[:], in1=m2t[:], op=ALU.mult
                        )
                        t_ = tp.tile([P, TW], F32, tag=f"v2{sfx}")
                        nc.vector.tensor_tensor(
                            out=t_[:, 1 : TW - 1], in0=mm_[:, 1 : TW - 1],
                            in1=crgt[:, 1 : TW - 1], op=ALU.mult,
                        )
                        ed = tp.tile([P, TW], F32, tag=f"v3{sfx}")
                        nc.vector.tensor_tensor(
                            out=ed[:, 1 : TW - 1], in0=highf[:, 1 : TW - 1],
                            in1=t_[:, 1 : TW - 1], op=ALU.add,
                        )
                        return ed

                    ed_m = finish(128, "m", vs_m, thin_m, high_m)
                    ed_x = finish(16, "x", vs_x, thin_x, high_x)

                    # bit-pack 8 rows per byte: pk[g, w] = sum_k ed[8g+k, w]<<k
                    pk = tp.tile([16, TW], U8, tag="pk")
                    for (lo, n) in TCHUNKS:
                        psP = tps.tile([16, 512], F32, tag="psP")
                        nc.tensor.matmul(
                            psP[:, :n], pack_m_s[:], ed_m[:, lo : lo + n],
                            start=True, stop=False,
                        )
                        nc.tensor.matmul(
                            psP[:, :n], pack_x_s[:], ed_x[:, lo : lo + n],
                            start=False, stop=True,
                        )
                        nc.vector.tensor_copy(pk[:, lo : lo + n], psP[:, :n])

                    for bb in range(B):
                        nc.sync.dma_start(
                            edges_d[bb, :, QW * Q : QW * Q + QW],
                            pk[:, QS * bb + 2 : QS * bb + 2 + QW],
                        )


_CTX = {}


def _get_ctx():
    if _CTX:
        return _CTX
    import jax
    from jax.sharding import Mesh, PartitionSpec, NamedSharding
    from jax.experimental.shard_map import shard_map
    from concourse import bass2jax

    nc = bacc.Bacc("TRN2", target_bir_lowering=False, debug=False,
                   num_devices=NCORES)
    _build(nc)
    nc.finalize()
    bass2jax.install_neuronx_cc_hook()

    partition_name = (
        nc.partition_id_tensor.name if nc.partition_id_tensor else None
    )
    in_names, out_names, out_avals = [], [], []
    for alloc in nc.m.functions[0].allocations:
        if not isinstance(alloc, mybir.MemoryLocationSet):
            continue
        name = alloc.memorylocations[0].name
        if alloc.kind == "ExternalInput":
            if name != partition_name:
                in_names.append(name)
        elif alloc.kind == "ExternalOutput":
            out_names.append(name)
            shape = tuple(alloc.tensor_shape)
            dtype = mybir.dt.np(alloc.dtype)
            out_avals.append(jax.core.ShapedArray(shape, dtype))
    n_params = len(in_names)
    n_outs = len(out_names)
    bind_in_names = list(in_names) + list(out_names)
    if partition_name is not None:
        bind_in_names.append(partition_name)
    bind_in_names = tuple(bind_in_names)

    def _body(*args):
        operands = list(args)
        if partition_name is not None:
            operands.append(bass2jax.partition_id_tensor())
        outs = bass2jax._bass_exec_p.bind(
            *operands,
            out_avals=tuple(out_avals),
            in_names=bind_in_names,
            out_names=tuple(out_names),
            lowering_input_output_aliases=(),
            sim_require_finite=True,
            sim_require_nnan=True,
            nc=nc,
        )
        return tuple(outs)

    devices = jax.devices()[:NCORES]
    mesh = Mesh(np.asarray(devices), ("core",))
    P = PartitionSpec
    donate = tuple(range(n_params, n_params + n_outs))
    sharded = jax.jit(
        shard_map(
            _body, mesh=mesh,
            in_specs=(P("core"),) * (n_params + n_outs),
            out_specs=(P("core"),) * n_outs,
            check_rep=False,
        ),
        donate_argnums=donate, keep_unused=True,
    )

    # on-device halo exchange: cores ship only their owned 128 rows
    # (48.0MB on the wire); 6-row halos come from neighbors via all_gather
    # of edge strips (ppermute is unsupported by this runtime).
    import jax.numpy as jnp

    def _reshard(x):  # local shard [C, B, 128, W] u16
        strips = jnp.stack([x[:, :, :6, :], x[:, :, -6:, :]], axis=0)
        g = jax.lax.all_gather(strips, "core")  # [8,2,C,B,6,W]
        i = jax.lax.axis_index("core")
        up = jnp.where(i > 0, g[jnp.maximum(i - 1, 0), 1], jnp.uint16(0))
        dn = jnp.where(i < NCORES - 1,
                       g[jnp.minimum(i + 1, NCORES - 1), 0], jnp.uint16(0))
        y = jnp.concatenate([up, x, dn], axis=2)  # [C,B,140,W]
        return y.reshape(C, B * BLK, W)

    reshard_jit = jax.jit(shard_map(
        _reshard, mesh=mesh, in_specs=(P("core"),), out_specs=P("core"),
        check_rep=False,
    ))

    # donated output buffers, created on-device (keeps the 1MB of zeros
    # off the wire; donation consumes them, so one call per kernel() run)
    zeros_jit = jax.jit(
        lambda: jnp.zeros((NCORES * B, 16, W), jnp.uint8),
        out_shardings=NamedSharding(mesh, P("core")),
    )

    # constant inputs, device-resident once
    core_sh = NamedSharding(mesh, P("core"))
    hm = np.zeros((NCORES * STACK, 1), np.float32)
    for core in range(NCORES):
        r0 = ROWS_PC * core
        for b in range(B):
            for pr in range(BLK):
                gr = r0 + pr - 6
                if 0 <= gr < H:
                    hm[core * STACK + b * BLK + pr, 0] = QSCALE
    hmask_dev = jax.device_put(hm, core_sh)
    hmask_dev.block_until_ready()

    dbg_zero = None
    if nc.dbg_addr is not None:
        dbg_zero = np.zeros((NCORES * 1, 2), np.uint32)

    _CTX.update(dict(
        jax=jax, nc=nc, mesh=mesh, core_sh=core_sh, devices=devices,
        sharded=sharded, reshard_jit=reshard_jit, zeros_jit=zeros_jit,
        in_names=in_names, out_names=out_names,
        n_params=n_params, hmask_dev=hmask_dev, dbg_zero=dbg_zero,
        dbg_name=nc.dbg_addr.name if nc.dbg_addr is not None else None,
        out_zero=np.zeros((NCORES * B, 16, W), np.uint8),
        slabs=[np.empty((C, B, ROWS_PC, W), np.uint16) for _ in range(NCORES)],
        tbuf=np.empty((C, B, ROWS_PC, W), np.float32),
        pool=ThreadPoolExecutor(C),
    ))
    return _CTX


def _build_chan(ctx, img, core, c):
    t = ctx["tbuf"][c]  # [B, 128, W] f32
    r0 = ROWS_PC * core
    np.multiply(img[:, c, r0 : r0 + ROWS_PC, :], QSCALE, out=t)
    np.rint(t, out=t)
    ctx["slabs"][core][c] = t


def _build_slab(ctx, img, core):
    """Quantize core's owned rows (img*256 -> u16), no halo."""
    list(ctx["pool"].map(lambda c: _build_chan(ctx, img, core, c), range(C)))
    return ctx["slabs"][core]


def kernel(img: np.ndarray) -> np.ndarray:
    img = np.asarray(img, dtype=np.float32)
    assert img.shape == (B, C, H, W)
    ctx = _get_ctx()
    jax = ctx["jax"]
    devices = ctx["devices"]

    out_zero_dev = ctx["zeros_jit"]()  # async, on-device, off the wire

    # build + ship per-core slabs; device_put runs on a worker thread so
    # quantization of slab c+1 overlaps the transfer of slab c.
    pieces = [None] * NCORES
    q = queue.Queue()

    def _xfer():
        while True:
            item = q.get()
            if item is None:
                return
            c, slab = item
            pieces[c] = jax.device_put(slab, devices[c])

    th = threading.Thread(target=_xfer)
    th.start()
    for core in range(NCORES):
        q.put((core, _build_slab(ctx, img, core)))
    q.put(None)
    th.join()

    img_own = jax.make_array_from_single_device_arrays(
        (NCORES * C, B, ROWS_PC, W), ctx["core_sh"], pieces
    )
    img_arr = ctx["reshard_jit"](img_own)  # async; chains into the bass jit

    args = []
    for name in ctx["in_names"]:
        if name == "img":
            args.append(img_arr)
        elif name == "hmask":
            args.append(ctx["hmask_dev"])
        elif name == ctx["dbg_name"]:
            args.append(ctx["dbg_zero"])
        else:
            raise KeyError(name)
    args.append(out_zero_dev)

    res = ctx["sharded"](*args)
    packed = np.asarray(res[0])  # [NCORES*B, 16, W] u8

    r = packed.reshape(NCORES, B, 16, W)
    out = np.empty((B, 1, H, W), np.float32)
    ov = out.reshape(B, NCORES, ROWS_PC, W)

    def _unp(b):  # out[b,0,128*core+r,w] = bit r%8 of packed[core,b,r//8,w]
        ov[b] = np.unpackbits(r[:, b], axis=1, bitorder="little")

    list(ctx["pool"].map(_unp, range(B)))
    out[..., 0, :] = 0.0
    out[..., -1, :] = 0.0
    out[..., :, 0] = 0.0
    out[..., :, -1] = 0.0
    return out


if __name__ == "__main__":
    rng = np.random.RandomState(0)
    x = (rng.rand(B, C, H, W) * 255).astype(np.float32)
    y = kernel(x)
    print("out", y.shape, y.mean())
